# revision 56
# baseline (speedup 1.0000x reference)
"""CLUB mutual-information upper bound (loss_fn) on 8 Trainium2 NeuronCores, v3.

Math: reference computes
    h  = relu(x1 @ W1 + b1); h = relu(h @ W2 + b2); g = tanh(h @ W3 + b3)
    mu, logvar = split(g); iv = exp(-logvar)
    pos = -0.5 (mu - x2)^2 iv
    neg = -0.5 mean_j[(mu_i - x2_j)^2] iv
    mi  = mean_i sum_d (pos - neg)

With m1 = mean_j x2, m2 = mean_j x2^2 (host-computed, global over all N):
    pos - neg = iv [ mu (x2 - m1) - 0.5 (x2^2 - m2) ] = iv (mu A - B)
Each core computes its 128-row shard's
    c1_d = sum_i iv*(mu*A),  c2_d = sum_i iv*B
and the host finishes mi = sum_cores sum_d (c1 - c2) / N.

v3 structure (from the v2 trace: exec = body_barrier_end + ~1.05us, with a
fixed ~7.3us postamble semaphore sweep inside the measured window):
  - biases ride as fp16 [128,1] columns at the head of the main blob and are
    applied via the ACT bias operand / DVE tensor_scalar scalar-AP operand,
    so no bias DMA and no bias matmuls gate L1 (v2 lost ~900ns to the bias
    DMA completing after x1/W1).
  - L3 writes both m-halves into ONE [128,256] psum tensor -> a single
    merged tanh produces mu|logvar together, exp queues back-to-back on
    scalar; b3 enters via two outer-product bias matmuls (1-partition rows
    from a tiny 1-descriptor DMA) placed in PE idle time before L1.
  - tail: tmp = mu*A (starts before iv lands), c2 = iv*B (accum),
    c1 = iv*tmp (accum): 3 DVE ops, last two after iv.
  - HWDGE stream: [brow] -> [bias|x1T|W1] (dw1) -> [W2] (dw2) -> [W3] (dw3),
    each completion ~data_end + 900ns; A|B (fp16 now) ride SWDGE.
  - warmup dummies sized to end ~dw1.
"""

import sys
from contextlib import ExitStack

import numpy as np

sys.path.insert(0, "/opt/trn_rl_repo")

import concourse.bass as bass
from concourse import mybir
from concourse.bass_utils import run_bass_kernel_spmd

F32 = mybir.dt.float32
F16 = mybir.dt.float16
NCORES = 8
N = 1024
X1D = 256
X2D = 128
HID = 256
ROWS = N // NCORES  # 128
P = 128

# blobW (fp16, SP ring) [128, 1168]:
#   [0:16)       bias cols: f32 bit-packed as fp16 pairs;
#                pair (2l+m)*2 = b_{l+1}[m*128:(m+1)*128]; pair 4 = zeros
#   [16:144)     x1T k0: col 16+j = x1s[j, p]
#   [144:400)    W1 k0: col 144 + m*128 + j = W1[p, m*128+j]
#   [400:912)    W2 full, k-major: col 400 + k*256 + m*128 + j
#   [912:1168)   W3 k0, lv|mu swapped: [912:1040) = W3[p, 128+j] (logvar),
#                [1040:1168) = W3[p, j] (mu) — logvar in the m0 psum so it
#                closes first and the scalar tanh->exp chain starts earlier
# blobA (fp16, ACT ring — streams CONCURRENTLY with the SP ring) [128, 640]:
#   [0:128)      x1T k1: col j = x1s[j, 128+p]
#   [128:384)    W1 k1: col 128 + m*128 + j = W1[128+p, m*128+j]
#   [384:640)    W3 k1, lv|mu swapped like W3 k0
# brow (fp16) [1, 384]: [0:128) b3 logvar half, [128:256) b3 mu half,
#                [256:384) ones
# blobAB (fp32) [128, 256]: [0:128) A = (x2s - m1).T ; [128:256) B = 0.5(x2s^2 - m2).T
#   (f32: the final value is a small difference of large sums, fp16 A/B cost
#    ~7e-3 of relative error on the result)
BIAS_OFF = 0
X1K0_OFF = 16
W1K0_OFF = 144
W2_OFF = 400
W3K0_OFF = 912
BW_W = 1168
BA_W = 640

N_DUMMIES = 18
N_NARROW = 4

_module_cache = None


def _build_module(n_dummies=N_DUMMIES, n_narrow=N_NARROW):
    nc = bass.Bass()
    blobW = nc.declare_dram_parameter("blobW", [P, BW_W], F16, isOutput=False)
    blobA = nc.declare_dram_parameter("blobA", [P, BA_W], F16, isOutput=False)
    brow = nc.declare_dram_parameter("brow", [1, 384], F16, isOutput=False)
    blobAB = nc.declare_dram_parameter("blobAB", [P, 256], F32, isOutput=False)
    out = nc.declare_dram_parameter("out", [P, 2], F32, isOutput=True)

    AF = mybir.ActivationFunctionType
    ALU = mybir.AluOpType

    with ExitStack() as ctx:
        ec = ctx.enter_context
        bw = ec(nc.sbuf_tensor("bw", [P, BW_W], F16))
        ba = ec(nc.sbuf_tensor("ba", [P, BA_W], F16))
        brow_sb = ec(nc.sbuf_tensor("brow_sb", [1, 384], F16))
        ab = ec(nc.sbuf_tensor("ab", [P, 256], F32))
        h1m0 = ec(nc.sbuf_tensor("h1m0", [P, ROWS], F16))
        h1m1 = ec(nc.sbuf_tensor("h1m1", [P, ROWS], F16))
        h2m0 = ec(nc.sbuf_tensor("h2m0", [P, ROWS], F16))
        h2m1 = ec(nc.sbuf_tensor("h2m1", [P, ROWS], F16))
        g = ec(nc.sbuf_tensor("g", [P, 2 * ROWS], F32))  # mu | logvar->tanh
        iv = ec(nc.sbuf_tensor("iv", [P, ROWS], F32))
        tmp = ec(nc.sbuf_tensor("tmp", [P, ROWS], F32))
        scr = ec(nc.sbuf_tensor("scr", [P, ROWS], F32))
        out_sb = ec(nc.sbuf_tensor("out_sb", [P, 2], F32))
        ps1m0 = ec(nc.psum_tensor("ps1m0", [P, ROWS], F32))
        ps1m1 = ec(nc.psum_tensor("ps1m1", [P, ROWS], F32))
        ps2m0 = ec(nc.psum_tensor("ps2m0", [P, ROWS], F32))
        ps2m1 = ec(nc.psum_tensor("ps2m1", [P, ROWS], F32))
        ps3m0 = ec(nc.psum_tensor("ps3m0", [P, ROWS], F32))
        ps3m1 = ec(nc.psum_tensor("ps3m1", [P, ROWS], F32))
        psw = ec(nc.psum_tensor("psw", [P, ROWS], F32))
        dbrow = ec(nc.semaphore("dbrow"))
        dw1a = ec(nc.semaphore("dw1a"))
        dax = ec(nc.semaphore("dax"))
        dw2 = ec(nc.semaphore("dw2"))
        dw3 = ec(nc.semaphore("dw3"))
        daw3 = ec(nc.semaphore("daw3"))
        dab = ec(nc.semaphore("dab"))
        s_pe = ec(nc.semaphore("s_pe"))
        s_act = ec(nc.semaphore("s_act"))
        s_dve = ec(nc.semaphore("s_dve"))
        dout = ec(nc.semaphore("dout"))
        block = ec(nc.Block())

        x1k0 = bw[:, X1K0_OFF : X1K0_OFF + 128]
        x1k1 = ba[:, 0:128]
        zcol = bw[:, 8:10].bitcast(F32)  # zero bias pair
        A_ap = ab[:, 0:ROWS]
        B_ap = ab[:, ROWS : 2 * ROWS]

        def b_col(l, m):
            c = BIAS_OFF + (2 * l + m) * 2
            return bw[:, c : c + 2].bitcast(F32)

        w1k0 = lambda m: bw[:, W1K0_OFF + m * 128 : W1K0_OFF + (m + 1) * 128]
        w1k1 = lambda m: ba[:, 128 + m * 128 : 128 + (m + 1) * 128]
        w2 = lambda k, m: bw[:, W2_OFF + k * 256 + m * 128 : W2_OFF + k * 256 + (m + 1) * 128]
        w3k0 = lambda m: bw[:, W3K0_OFF + m * 128 : W3K0_OFF + (m + 1) * 128]
        w3k1 = lambda m: ba[:, 384 + m * 128 : 384 + (m + 1) * 128]

        b3lv_row = brow_sb[0:1, 0:128]
        b3mu_row = brow_sb[0:1, 128:256]
        ones_row = brow_sb[0:1, 256:384]

        mu_ap = g[:, 0:ROWS]
        lv_ap = g[:, ROWS : 2 * ROWS]

        @block.sync
        def _(sync):
            # SP-ring HWDGE stream: [bias|x1k0|W1k0] -> [W2] -> [W3k0]. The
            # k1 halves stream CONCURRENTLY on the ACT ring (scalar engine),
            # roughly doubling effective input bandwidth.
            sync.dma_start(
                out=bw[:, 0:W2_OFF], in_=blobW[:, 0:W2_OFF]
            ).then_inc(dw1a, 16)
            sync.dma_start(
                out=bw[:, W2_OFF:W3K0_OFF], in_=blobW[:, W2_OFF:W3K0_OFF]
            ).then_inc(dw2, 16)
            sync.dma_start(
                out=bw[:, W3K0_OFF:BW_W], in_=blobW[:, W3K0_OFF:BW_W]
            ).then_inc(dw3, 16)
            # out DMA released when exp is done (s_act>=4): the ~640ns
            # issue + the idle-queue first-read latency (~770ns) covers the
            # trailing mu-tanh + DVE accum writes with ~600ns margin, and
            # the fixed end-of-kernel semaphore sweep covers the flight.
            sync.wait_ge(s_act, 4)
            sync.dma_start(out=out[:], in_=out_sb[:]).then_inc(dout, 16)

        @block.gpsimd
        def _(gpsimd):
            # SWDGE path: the tiny b3/ones row (needed by the L3 bias
            # matmuls mid-chain), then the f32 A|B (tail only).
            gpsimd.dma_start(out=brow_sb[0:1, :], in_=brow[0:1, :]).then_inc(dbrow, 16)
            gpsimd.dma_start(out=ab[:], in_=blobAB[:]).then_inc(dab, 16)

        @block.scalar
        def _(scalar):
            # ACT-ring DMAs first: [x1k1|W1k1] then [W3k1], streaming in
            # parallel with the SP ring.
            scalar.dma_start(out=ba[:, 0:384], in_=blobA[:, 0:384]).then_inc(dax, 16)
            scalar.dma_start(out=ba[:, 384:BA_W], in_=blobA[:, 384:BA_W]).then_inc(daw3, 16)
            # dummy activations: ACT table load (relu/tanh/exp) starts early.
            # All biases are APs (zcol holds garbage this early; the dummy
            # outputs are dead stores into scr).
            scalar.activation(
                out=scr[0:1, 0:1], in_=scr[0:1, 0:1], func=AF.Relu,
                bias=zcol[0:1, 0:1], scale=1.0
            )
            scalar.activation(
                out=scr[0:1, 0:1], in_=scr[0:1, 0:1], func=AF.Tanh,
                bias=zcol[0:1, 0:1], scale=1.0
            )
            scalar.activation(
                out=scr[0:1, 0:1], in_=scr[0:1, 0:1], func=AF.Exp,
                bias=zcol[0:1, 0:1], scale=0.0
            )
            # relu m1 halves (bias via ACT bias operand, f32 [128,1] AP)
            scalar.wait_ge(s_pe, 2)
            scalar.activation(
                out=h1m1[:], in_=ps1m1[:], func=AF.Relu, bias=b_col(0, 1), scale=1.0
            ).then_inc(s_act)
            scalar.wait_ge(s_pe, 4)
            scalar.activation(
                out=h2m1[:], in_=ps2m1[:], func=AF.Relu, bias=b_col(1, 1), scale=1.0
            ).then_inc(s_act)
            # logvar half (ps3m0, closes first) feeds exp back-to-back; the
            # mu tanh follows
            scalar.wait_ge(s_pe, 5)
            scalar.activation(
                out=lv_ap, in_=ps3m0[:], func=AF.Tanh, bias=zcol, scale=1.0
            ).then_inc(s_act)
            scalar.activation(
                out=iv[:], in_=lv_ap, func=AF.Exp, bias=zcol, scale=-1.0
            ).then_inc(s_act)
            scalar.wait_ge(s_pe, 6)
            scalar.activation(
                out=mu_ap, in_=ps3m1[:], func=AF.Tanh, bias=zcol, scale=1.0
            ).then_inc(s_act)

        @block.tensor
        def _(tensor):
            # warmup: keep the PE busy while the input DMA flies.
            for _i in range(n_dummies):
                tensor.matmul(psw[:], lhsT=bw[:, 0:128], rhs=bw[:, 0:128],
                              start=True, stop=True)
            for _i in range(n_narrow):
                tensor.matmul(psw[:, 0:32], lhsT=bw[:, 0:128],
                              rhs=bw[:, 0:32], start=True, stop=True)
            # L1: bias comes via the relu, so psum groups start at k0.
            # k0 operands ride the SP ring, k1 operands the ACT ring.
            tensor.wait_ge(dw1a, 16)
            tensor.matmul(ps1m0[:], lhsT=w1k0(0), rhs=x1k0, start=True, stop=False)
            tensor.matmul(ps1m1[:], lhsT=w1k0(1), rhs=x1k0, start=True, stop=False)
            tensor.wait_ge(dax, 16)
            tensor.matmul(ps1m0[:], lhsT=w1k1(0), rhs=x1k1, start=False, stop=True).then_inc(s_pe)
            tensor.matmul(ps1m1[:], lhsT=w1k1(1), rhs=x1k1, start=False, stop=True).then_inc(s_pe)
            # prefetch-wait: dw2 lands while PE waits for the relus
            tensor.wait_ge(dw2, 16)
            # L2: k0 needs h1m0 (DVE), k1 needs h1m1 (ACT)
            tensor.wait_ge(s_dve, 1)
            tensor.matmul(ps2m0[:], lhsT=w2(0, 0), rhs=h1m0[:], start=True, stop=False)
            tensor.matmul(ps2m1[:], lhsT=w2(0, 1), rhs=h1m0[:], start=True, stop=False)
            tensor.wait_ge(s_act, 1)
            tensor.matmul(ps2m0[:], lhsT=w2(1, 0), rhs=h1m1[:], start=False, stop=True).then_inc(s_pe)
            tensor.matmul(ps2m1[:], lhsT=w2(1, 1), rhs=h1m1[:], start=False, stop=True).then_inc(s_pe)
            # prefetch-waits (all land well before relu2 finishes); the b3
            # bias matmuls open both L3 psum banks in this idle window (the
            # ps2 groups are closed by now).
            tensor.wait_ge(dw3, 16)
            tensor.wait_ge(daw3, 16)
            tensor.wait_ge(dbrow, 16)
            tensor.matmul(ps3m0[:], lhsT=b3lv_row, rhs=ones_row,
                          start=True, stop=False)
            tensor.matmul(ps3m1[:], lhsT=b3mu_row, rhs=ones_row,
                          start=True, stop=False)
            # L3: ps3m0 = logvar (closes first -> scalar chain starts one
            # matmul earlier), ps3m1 = mu.
            tensor.wait_ge(s_dve, 2)
            tensor.matmul(ps3m0[:], lhsT=w3k0(0), rhs=h2m0[:], start=False, stop=False)
            tensor.matmul(ps3m1[:], lhsT=w3k0(1), rhs=h2m0[:], start=False, stop=False)
            tensor.wait_ge(s_act, 2)
            tensor.matmul(ps3m0[:], lhsT=w3k1(0), rhs=h2m1[:], start=False, stop=True).then_inc(s_pe)
            tensor.matmul(ps3m1[:], lhsT=w3k1(1), rhs=h2m1[:], start=False, stop=True).then_inc(s_pe)

        @block.vector
        def _(vector):
            # relu m0 halves: (ps + bias) then max 0, one DVE op each
            vector.wait_ge(s_pe, 1)
            vector.tensor_scalar(
                out=h1m0[:], in0=ps1m0[:], scalar1=b_col(0, 0), scalar2=0.0,
                op0=ALU.add, op1=ALU.max,
            ).then_inc(s_dve)
            vector.wait_ge(s_pe, 3)
            vector.tensor_scalar(
                out=h2m0[:], in0=ps2m0[:], scalar1=b_col(1, 0), scalar2=0.0,
                op0=ALU.add, op1=ALU.max,
            ).then_inc(s_dve)
            # prefetch-wait: dab lands long before the tail needs A|B
            vector.wait_ge(dab, 16)
            # tail: c2 = iv*B as soon as exp lands, tmp = mu*A once the mu
            # tanh lands, then c1 = iv*tmp; accumulators give the
            # per-partition sums.
            vector.wait_ge(s_act, 4)
            vector.scalar_tensor_tensor(
                out=scr[:], in0=iv[:], scalar=1.0, in1=B_ap,
                op0=ALU.bypass, op1=ALU.mult, accum_out=out_sb[:, 1:2],
            )
            vector.wait_ge(s_act, 5)
            vector.scalar_tensor_tensor(
                out=tmp[:], in0=mu_ap, scalar=1.0, in1=A_ap,
                op0=ALU.bypass, op1=ALU.mult,
            )
            vector.scalar_tensor_tensor(
                out=scr[:], in0=iv[:], scalar=1.0, in1=tmp[:],
                op0=ALU.bypass, op1=ALU.mult, accum_out=out_sb[:, 0:1],
            ).then_inc(s_dve)

    _split_multi_waits(nc)
    return nc


def _split_multi_waits(nc):
    """This walrus build encodes at most one sync-wait per instruction.
    Hoist extra waits onto same-engine NoOps immediately preceding the
    instruction (engines execute their stream in order, so this is
    semantically identical)."""
    for fn in nc.m.functions:
        for bb in fn.blocks:
            new_insts = []
            for ins in bb.instructions:
                si = ins.sync_info
                if si is not None and len(si.on_wait) > 1:
                    waits = list(si.on_wait)
                    for j, w in enumerate(waits[:-1]):
                        nop = mybir.InstNoOp(
                            name=f"{ins.name}-sw{j}",
                            sync_info=mybir.SyncInfo(on_wait=[w], on_update=[]),
                            bass_nofuse=True,
                            engine=ins.engine,
                        )
                        new_insts.append(nop)
                    si.on_wait = [waits[-1]]
                new_insts.append(ins)
            if len(new_insts) != len(bb.instructions):
                bb.instructions[:] = new_insts


def _pack_inputs(x1, x2, W1, b1, W2, b2, W3, b3):
    f32, f16 = np.float32, np.float16

    W1 = np.ascontiguousarray(W1, f32)
    W2 = np.ascontiguousarray(W2, f32)
    W3 = np.ascontiguousarray(W3, f32)
    b1 = np.asarray(b1, f32)
    b2 = np.asarray(b2, f32)
    b3 = np.asarray(b3, f32)
    brow = np.zeros((1, 384), f16)
    brow[0, 0:128] = b3[128:256].astype(f16)   # logvar half -> ps3m0
    brow[0, 128:256] = b3[0:128].astype(f16)   # mu half -> ps3m1
    brow[0, 256:384] = 1.0
    x2f = np.asarray(x2, np.float64)
    m1 = x2f.mean(0)
    m2 = (x2f * x2f).mean(0)

    # shared weight sections
    bwc = np.empty((P, BW_W), f16)  # per-core copy made below (x1 differs)
    for m in range(2):
        bwc[:, W1K0_OFF + m * 128 : W1K0_OFF + (m + 1) * 128] = W1[
            0:128, m * 128 : (m + 1) * 128
        ].astype(f16)
    for k in range(2):
        for m in range(2):
            bwc[:, W2_OFF + k * 256 + m * 128 : W2_OFF + k * 256 + (m + 1) * 128] = W2[
                k * 128 : (k + 1) * 128, m * 128 : (m + 1) * 128
            ].astype(f16)
    # W3 k0, lv|mu swapped
    bwc[:, W3K0_OFF : W3K0_OFF + 128] = W3[0:128, 128:256].astype(f16)
    bwc[:, W3K0_OFF + 128 : W3K0_OFF + 256] = W3[0:128, 0:128].astype(f16)
    bw_u16 = bwc.view(np.uint16)
    for li, b in enumerate((b1, b2)):
        for m in range(2):
            c = (2 * li + m) * 2
            bw_u16[:, c : c + 2] = (
                b[m * 128 : (m + 1) * 128].astype(f32).view(np.uint16).reshape(P, 2)
            )
    bw_u16[:, 8:16] = 0

    bac = np.empty((P, BA_W), f16)
    for m in range(2):
        bac[:, 128 + m * 128 : 128 + (m + 1) * 128] = W1[
            128:256, m * 128 : (m + 1) * 128
        ].astype(f16)
    bac[:, 384:512] = W3[128:256, 128:256].astype(f16)  # lv half
    bac[:, 512:640] = W3[128:256, 0:128].astype(f16)    # mu half

    in_maps = []
    for c in range(NCORES):
        x1s = np.asarray(x1[c * ROWS : (c + 1) * ROWS], f32)
        x2s = np.asarray(x2[c * ROWS : (c + 1) * ROWS], np.float64)
        bw = bwc.copy()
        bw[:, X1K0_OFF : X1K0_OFF + 128] = x1s[:, 0:128].T.astype(f16)
        ba = bac.copy()
        ba[:, 0:128] = x1s[:, 128:256].T.astype(f16)
        abb = np.empty((P, 256), f32)
        abb[:, 0:ROWS] = (x2s - m1).T.astype(f32)
        abb[:, ROWS : 2 * ROWS] = (0.5 * (x2s * x2s - m2)).T.astype(f32)
        in_maps.append({"blobW": bw, "blobA": ba, "brow": brow, "blobAB": abb})
    return in_maps


def _run(in_maps, **kwargs):
    global _module_cache
    if _module_cache is None:
        _module_cache = _build_module()
    return run_bass_kernel_spmd(
        _module_cache, in_maps, core_ids=list(range(NCORES)), **kwargs
    )


def _combine(results):
    tot = 0.0
    for r in results:
        o = np.asarray(r["out"], np.float64)
        tot += float(np.sum(o[:, 0] - o[:, 1]))
    return np.float32(tot / N)


def kernel(x1, x2, W1, b1, W2, b2, W3, b3):
    in_maps = _pack_inputs(x1, x2, W1, b1, W2, b2, W3, b3)
    res = _run(in_maps)
    return _combine(res.results)


# revision 58
# speedup vs baseline: 1.0221x; 1.0221x over previous
"""CLUB mutual-information upper bound (loss_fn) on 8 Trainium2 NeuronCores, v3.

Math: reference computes
    h  = relu(x1 @ W1 + b1); h = relu(h @ W2 + b2); g = tanh(h @ W3 + b3)
    mu, logvar = split(g); iv = exp(-logvar)
    pos = -0.5 (mu - x2)^2 iv
    neg = -0.5 mean_j[(mu_i - x2_j)^2] iv
    mi  = mean_i sum_d (pos - neg)

With m1 = mean_j x2, m2 = mean_j x2^2 (host-computed, global over all N):
    pos - neg = iv [ mu (x2 - m1) - 0.5 (x2^2 - m2) ] = iv (mu A - B)
Each core computes its 128-row shard's
    c1_d = sum_i iv*(mu*A),  c2_d = sum_i iv*B
and the host finishes mi = sum_cores sum_d (c1 - c2) / N.

v3 structure (from the v2 trace: exec = body_barrier_end + ~1.05us, with a
fixed ~7.3us postamble semaphore sweep inside the measured window):
  - biases ride as fp16 [128,1] columns at the head of the main blob and are
    applied via the ACT bias operand / DVE tensor_scalar scalar-AP operand,
    so no bias DMA and no bias matmuls gate L1 (v2 lost ~900ns to the bias
    DMA completing after x1/W1).
  - L3 writes both m-halves into ONE [128,256] psum tensor -> a single
    merged tanh produces mu|logvar together, exp queues back-to-back on
    scalar; b3 enters via two outer-product bias matmuls (1-partition rows
    from a tiny 1-descriptor DMA) placed in PE idle time before L1.
  - tail: tmp = mu*A (starts before iv lands), c2 = iv*B (accum),
    c1 = iv*tmp (accum): 3 DVE ops, last two after iv.
  - HWDGE stream: [brow] -> [bias|x1T|W1] (dw1) -> [W2] (dw2) -> [W3] (dw3),
    each completion ~data_end + 900ns; A|B (fp16 now) ride SWDGE.
  - warmup dummies sized to end ~dw1.
"""

import sys
from contextlib import ExitStack

import numpy as np

sys.path.insert(0, "/opt/trn_rl_repo")

import concourse.bass as bass
from concourse import mybir
from concourse.bass_utils import run_bass_kernel_spmd

F32 = mybir.dt.float32
F16 = mybir.dt.float16
NCORES = 8
N = 1024
X1D = 256
X2D = 128
HID = 256
ROWS = N // NCORES  # 128
P = 128

# blobW (fp16) [128, 1808]:
#   [0:16)       bias cols: f32 values bit-packed as fp16 pairs;
#                pair (2l+m)*2 = b_{l+1}[m*128:(m+1)*128]; pair 4 = zeros
#                (AP biases everywhere keep the framework from emitting
#                const-AP memsets in the preamble)
#   [16:272)     x1T   col 16 + k*128 + j = x1s[j, k*128+p]
#   [272:784)    W1    col 272 + k*256 + m*128 + j = W1[k*128+p, m*128+j]
#                (k-major so [bias|x1|W1k0] is one contiguous DMA -> dw1a,
#                 W1k1 a second -> dw1b: k0 matmuls start one DMA earlier)
#   [784:1040)   W2 k0 only
#   [1040:1296)  W3 k0 only — the k1 halves of W2 and W3 ride the SWDGE
#                path: the HWDGE ring streams only ~140-165GB/s, so
#                shedding 132KB pulls dw2 in before relu1 finishes
# blobW2K1/blobW3K1 (fp16) [128, 256]: k1 halves, col m*128+j = W[128+p, m*128+j]
# brow (fp16) [1, 384]: [0:128) b3m0, [128:256) b3m1, [256:384) ones
# blobAB (fp32) [128, 256]: [0:128) A = (x2s - m1).T ; [128:256) B = 0.5(x2s^2 - m2).T
#   (f32: the final value is a small difference of large sums, fp16 A/B cost
#    ~7e-3 of relative error on the result)
BIAS_OFF = 0
X1_OFF = 16
W1_OFF = 272
W1K1_OFF = 528
W2_OFF = 784
W3_OFF = 1040
BW_W = 1296

N_DUMMIES = 18
N_NARROW = 4

_module_cache = None


def _build_module(n_dummies=N_DUMMIES, n_narrow=N_NARROW):
    nc = bass.Bass()
    blobW = nc.declare_dram_parameter("blobW", [P, BW_W], F16, isOutput=False)
    blobW2K1 = nc.declare_dram_parameter("blobW2K1", [P, 256], F16, isOutput=False)
    blobW3K1 = nc.declare_dram_parameter("blobW3K1", [P, 256], F16, isOutput=False)
    brow = nc.declare_dram_parameter("brow", [1, 384], F16, isOutput=False)
    blobAB = nc.declare_dram_parameter("blobAB", [P, 256], F32, isOutput=False)
    out = nc.declare_dram_parameter("out", [P, 2], F32, isOutput=True)

    AF = mybir.ActivationFunctionType
    ALU = mybir.AluOpType

    with ExitStack() as ctx:
        ec = ctx.enter_context
        bw = ec(nc.sbuf_tensor("bw", [P, BW_W], F16))
        w2k1 = ec(nc.sbuf_tensor("w2k1", [P, 256], F16))
        w3k1 = ec(nc.sbuf_tensor("w3k1", [P, 256], F16))
        brow_sb = ec(nc.sbuf_tensor("brow_sb", [1, 384], F16))
        ab = ec(nc.sbuf_tensor("ab", [P, 256], F32))
        h1m0 = ec(nc.sbuf_tensor("h1m0", [P, ROWS], F16))
        h1m1 = ec(nc.sbuf_tensor("h1m1", [P, ROWS], F16))
        h2m0 = ec(nc.sbuf_tensor("h2m0", [P, ROWS], F16))
        h2m1 = ec(nc.sbuf_tensor("h2m1", [P, ROWS], F16))
        g = ec(nc.sbuf_tensor("g", [P, 2 * ROWS], F32))  # mu | logvar->tanh
        iv = ec(nc.sbuf_tensor("iv", [P, ROWS], F32))
        tmp = ec(nc.sbuf_tensor("tmp", [P, ROWS], F32))
        scr = ec(nc.sbuf_tensor("scr", [P, ROWS], F32))
        out_sb = ec(nc.sbuf_tensor("out_sb", [P, 2], F32))
        ps1m0 = ec(nc.psum_tensor("ps1m0", [P, ROWS], F32))
        ps1m1 = ec(nc.psum_tensor("ps1m1", [P, ROWS], F32))
        ps2m0 = ec(nc.psum_tensor("ps2m0", [P, ROWS], F32))
        ps2m1 = ec(nc.psum_tensor("ps2m1", [P, ROWS], F32))
        ps3m0 = ec(nc.psum_tensor("ps3m0", [P, ROWS], F32))
        ps3m1 = ec(nc.psum_tensor("ps3m1", [P, ROWS], F32))
        psw = ec(nc.psum_tensor("psw", [P, ROWS], F32))
        dbrow = ec(nc.semaphore("dbrow"))
        dw1a = ec(nc.semaphore("dw1a"))
        dw1b = ec(nc.semaphore("dw1b"))
        dw2 = ec(nc.semaphore("dw2"))
        dw3 = ec(nc.semaphore("dw3"))
        dw2k1 = ec(nc.semaphore("dw2k1"))
        dw3k1 = ec(nc.semaphore("dw3k1"))
        dab = ec(nc.semaphore("dab"))
        s_pe = ec(nc.semaphore("s_pe"))
        s_act = ec(nc.semaphore("s_act"))
        s_dve = ec(nc.semaphore("s_dve"))
        dout = ec(nc.semaphore("dout"))
        block = ec(nc.Block())

        x1T = [bw[:, X1_OFF : X1_OFF + 128], bw[:, X1_OFF + 128 : X1_OFF + 256]]
        zcol = bw[:, 8:10].bitcast(F32)  # zero bias pair
        A_ap = ab[:, 0:ROWS]
        B_ap = ab[:, ROWS : 2 * ROWS]

        def w_ap(woff, k, m):
            c = woff + k * 256 + m * 128
            return bw[:, c : c + 128]

        def b_col(l, m):
            c = BIAS_OFF + (2 * l + m) * 2
            return bw[:, c : c + 2].bitcast(F32)

        b3m0_row = brow_sb[0:1, 0:128]
        b3m1_row = brow_sb[0:1, 128:256]
        ones_row = brow_sb[0:1, 256:384]

        mu_ap = g[:, 0:ROWS]
        lv_ap = g[:, ROWS : 2 * ROWS]

        @block.sync
        def _(sync):
            # HWDGE stream: the L1-critical prefix first ([bias|x1|W1k0] then
            # [W1k1] so k0 matmuls start one DMA-chunk earlier), then W2, W3
            # pipelining behind on the same ring.
            sync.dma_start(
                out=bw[:, 0:W1K1_OFF], in_=blobW[:, 0:W1K1_OFF]
            ).then_inc(dw1a, 16)
            sync.dma_start(
                out=bw[:, W1K1_OFF:W2_OFF], in_=blobW[:, W1K1_OFF:W2_OFF]
            ).then_inc(dw1b, 16)
            sync.dma_start(
                out=bw[:, W2_OFF:W3_OFF], in_=blobW[:, W2_OFF:W3_OFF]
            ).then_inc(dw2, 16)
            sync.dma_start(
                out=bw[:, W3_OFF:BW_W], in_=blobW[:, W3_OFF:BW_W]
            ).then_inc(dw3, 16)  # W3 k0 only
            # out DMA released when exp is done (s_act>=4): the ~640ns
            # issue + the idle-queue first-read latency (~770ns) covers the
            # trailing mu-tanh + DVE accum writes with ~600ns margin, and
            # the fixed end-of-kernel semaphore sweep covers the flight.
            sync.wait_ge(s_act, 4)
            sync.dma_start(out=out[:], in_=out_sb[:]).then_inc(dout, 16)

        @block.gpsimd
        def _(gpsimd):
            # SWDGE has its own descriptor generator and runs in parallel
            # with the HWDGE ring: W3's k1 half first (sheds 66KB off the
            # HWDGE stream), then the tiny b3/ones row (needed by the L3
            # bias matmuls mid-chain), then the f32 A|B (tail only).
            # (x1 via SWDGE was tried and lost ~700ns: the Pool engine
            # issues its first DMA ~650ns later than Sync does.)
            gpsimd.dma_start(out=w2k1[:], in_=blobW2K1[:]).then_inc(dw2k1, 16)
            gpsimd.dma_start(out=brow_sb[0:1, :], in_=brow[0:1, :]).then_inc(dbrow, 16)
            gpsimd.dma_start(out=w3k1[:], in_=blobW3K1[:]).then_inc(dw3k1, 16)
            gpsimd.dma_start(out=ab[:], in_=blobAB[:]).then_inc(dab, 16)

        @block.scalar
        def _(scalar):
            # dummy activations: ACT table load (relu/tanh/exp) starts early.
            # All biases are APs (zcol) so the framework emits no const-AP
            # memsets in the preamble (zcol holds garbage this early; the
            # dummy outputs are dead stores into scr).
            scalar.activation(
                out=scr[0:1, 0:1], in_=scr[0:1, 0:1], func=AF.Relu,
                bias=zcol[0:1, 0:1], scale=1.0
            )
            scalar.activation(
                out=scr[0:1, 0:1], in_=scr[0:1, 0:1], func=AF.Tanh,
                bias=zcol[0:1, 0:1], scale=1.0
            )
            scalar.activation(
                out=scr[0:1, 0:1], in_=scr[0:1, 0:1], func=AF.Exp,
                bias=zcol[0:1, 0:1], scale=0.0
            )
            # relu m1 halves (bias via ACT bias operand, fp16 [128,1] AP)
            scalar.wait_ge(s_pe, 2)
            scalar.activation(
                out=h1m1[:], in_=ps1m1[:], func=AF.Relu, bias=b_col(0, 1), scale=1.0
            ).then_inc(s_act)
            scalar.wait_ge(s_pe, 4)
            scalar.activation(
                out=h2m1[:], in_=ps2m1[:], func=AF.Relu, bias=b_col(1, 1), scale=1.0
            ).then_inc(s_act)
            # logvar lives in ps3m0, which closes one matmul earlier than
            # ps3m1 -> the tanh->exp chain starts ~107ns sooner. The mu tanh
            # reads ps3m1 without a wait: k1m1 retires <=110ns after k1m0
            # while tanh_lv+exp occupy scalar >=650ns.
            scalar.wait_ge(s_pe, 5)
            scalar.activation(
                out=lv_ap, in_=ps3m0[:], func=AF.Tanh, bias=zcol, scale=1.0
            ).then_inc(s_act)
            scalar.activation(
                out=iv[:], in_=lv_ap, func=AF.Exp, bias=zcol, scale=-1.0
            ).then_inc(s_act)
            scalar.activation(
                out=mu_ap, in_=ps3m1[:], func=AF.Tanh, bias=zcol, scale=1.0
            ).then_inc(s_act)

        @block.tensor
        def _(tensor):
            # warmup: keep the PE clock ramped while the input DMA flies.
            for _i in range(n_dummies):
                tensor.matmul(psw[:], lhsT=bw[:, 0:128], rhs=bw[:, 0:128],
                              start=True, stop=True)
            for _i in range(n_narrow):
                tensor.matmul(psw[:, 0:32], lhsT=bw[:, 0:128],
                              rhs=bw[:, 0:32], start=True, stop=True)
            # L1: bias comes via the relu, so psum groups start at k0.
            # [bias|x1|W1k0] arrive one HWDGE chunk before W1k1.
            tensor.wait_ge(dw1a, 16)
            tensor.matmul(ps1m0[:], lhsT=w_ap(W1_OFF, 0, 0), rhs=x1T[0], start=True, stop=False)
            tensor.matmul(ps1m1[:], lhsT=w_ap(W1_OFF, 0, 1), rhs=x1T[0], start=True, stop=False)
            tensor.wait_ge(dw1b, 16)
            tensor.matmul(ps1m0[:], lhsT=w_ap(W1_OFF, 1, 0), rhs=x1T[1], start=False, stop=True).then_inc(s_pe)
            tensor.matmul(ps1m1[:], lhsT=w_ap(W1_OFF, 1, 1), rhs=x1T[1], start=False, stop=True).then_inc(s_pe)
            # Prefetch-waits: these DMA semaphores land while PE sits idle
            # waiting for the relus, so waiting here (instead of adjacent to
            # the dependent matmuls) takes the ~110ns/wait sequencer cost
            # off the critical path.
            tensor.wait_ge(dw2k1, 16)
            tensor.wait_ge(dw2, 16)
            # L2: k0 needs h1m0 (DVE), k1 needs h1m1 (ACT)
            tensor.wait_ge(s_dve, 1)
            tensor.matmul(ps2m0[:], lhsT=w_ap(W2_OFF, 0, 0), rhs=h1m0[:], start=True, stop=False)
            tensor.matmul(ps2m1[:], lhsT=w_ap(W2_OFF, 0, 1), rhs=h1m0[:], start=True, stop=False)
            tensor.wait_ge(s_act, 1)
            tensor.matmul(ps2m0[:], lhsT=w2k1[:, 0:128], rhs=h1m1[:], start=False, stop=True).then_inc(s_pe)
            tensor.matmul(ps2m1[:], lhsT=w2k1[:, 128:256], rhs=h1m1[:], start=False, stop=True).then_inc(s_pe)
            # prefetch-waits (all land well before relu2 finishes)
            tensor.wait_ge(dw3, 16)
            tensor.wait_ge(dw3k1, 16)
            tensor.wait_ge(dbrow, 16)
            # The b3 bias matmuls open both L3 psum banks here, in the PE
            # idle window while the relu2 pair runs (the ps2 groups are
            # closed by now, so at most two accumulation groups are open).
            tensor.matmul(ps3m0[:], lhsT=b3m0_row, rhs=ones_row,
                          start=True, stop=False)
            tensor.matmul(ps3m1[:], lhsT=b3m1_row, rhs=ones_row,
                          start=True, stop=False)
            # L3: the four k-matmuls accumulate; m0 (mu) stops first, m1
            # (logvar) right behind.
            tensor.wait_ge(s_dve, 2)
            tensor.matmul(ps3m0[:], lhsT=w_ap(W3_OFF, 0, 0), rhs=h2m0[:], start=False, stop=False)
            tensor.matmul(ps3m1[:], lhsT=w_ap(W3_OFF, 0, 1), rhs=h2m0[:], start=False, stop=False)
            tensor.wait_ge(s_act, 2)
            tensor.matmul(ps3m0[:], lhsT=w3k1[:, 0:128], rhs=h2m1[:], start=False, stop=True).then_inc(s_pe)
            tensor.matmul(ps3m1[:], lhsT=w3k1[:, 128:256], rhs=h2m1[:], start=False, stop=True).then_inc(s_pe)

        @block.vector
        def _(vector):
            # relu m0 halves: (ps + bias) then max 0, one DVE op each
            vector.wait_ge(s_pe, 1)
            vector.tensor_scalar(
                out=h1m0[:], in0=ps1m0[:], scalar1=b_col(0, 0), scalar2=0.0,
                op0=ALU.add, op1=ALU.max,
            ).then_inc(s_dve)
            vector.wait_ge(s_pe, 3)
            vector.tensor_scalar(
                out=h2m0[:], in0=ps2m0[:], scalar1=b_col(1, 0), scalar2=0.0,
                op0=ALU.add, op1=ALU.max,
            ).then_inc(s_dve)
            # prefetch-wait: dab lands long before the tail needs A|B
            vector.wait_ge(dab, 16)
            # tail: c2 = iv*B as soon as exp lands, tmp = mu*A once the mu
            # tanh lands, then c1 = iv*tmp; accumulators give the
            # per-partition sums.
            vector.wait_ge(s_act, 4)
            vector.scalar_tensor_tensor(
                out=scr[:], in0=iv[:], scalar=1.0, in1=B_ap,
                op0=ALU.bypass, op1=ALU.mult, accum_out=out_sb[:, 1:2],
            )
            vector.wait_ge(s_act, 5)
            vector.scalar_tensor_tensor(
                out=tmp[:], in0=mu_ap, scalar=1.0, in1=A_ap,
                op0=ALU.bypass, op1=ALU.mult,
            )
            vector.scalar_tensor_tensor(
                out=scr[:], in0=iv[:], scalar=1.0, in1=tmp[:],
                op0=ALU.bypass, op1=ALU.mult, accum_out=out_sb[:, 0:1],
            ).then_inc(s_dve)

    _split_multi_waits(nc)
    return nc


def _split_multi_waits(nc):
    """This walrus build encodes at most one sync-wait per instruction.
    Hoist extra waits onto same-engine NoOps immediately preceding the
    instruction (engines execute their stream in order, so this is
    semantically identical)."""
    for fn in nc.m.functions:
        for bb in fn.blocks:
            new_insts = []
            for ins in bb.instructions:
                si = ins.sync_info
                if si is not None and len(si.on_wait) > 1:
                    waits = list(si.on_wait)
                    for j, w in enumerate(waits[:-1]):
                        nop = mybir.InstNoOp(
                            name=f"{ins.name}-sw{j}",
                            sync_info=mybir.SyncInfo(on_wait=[w], on_update=[]),
                            bass_nofuse=True,
                            engine=ins.engine,
                        )
                        new_insts.append(nop)
                    si.on_wait = [waits[-1]]
                new_insts.append(ins)
            if len(new_insts) != len(bb.instructions):
                bb.instructions[:] = new_insts


def _pack_inputs(x1, x2, W1, b1, W2, b2, W3, b3):
    f32, f16 = np.float32, np.float16

    def wsec(W):
        W = np.ascontiguousarray(W, f32)
        s = np.empty((P, 512), f16)
        for k in range(2):
            for m in range(2):
                s[:, k * 256 + m * 128 : k * 256 + (m + 1) * 128] = W[
                    k * 128 : (k + 1) * 128, m * 128 : (m + 1) * 128
                ].astype(f16)
        return s

    w1s, w2s, w3s = wsec(W1), wsec(W2), wsec(W3)
    b1 = np.asarray(b1, f32)
    b2 = np.asarray(b2, f32)
    b3 = np.asarray(b3, f32)
    brow = np.zeros((1, 384), f16)
    brow[0, 0:128] = b3[128:256].astype(f16)   # logvar half -> ps3m0
    brow[0, 128:256] = b3[0:128].astype(f16)   # mu half -> ps3m1
    brow[0, 256:384] = 1.0
    x2f = np.asarray(x2, np.float64)
    m1 = x2f.mean(0)
    m2 = (x2f * x2f).mean(0)
    in_maps = []
    for c in range(NCORES):
        bw = np.empty((P, BW_W), f16)
        x1s = np.asarray(x1[c * ROWS : (c + 1) * ROWS], f32)
        x2s = np.asarray(x2[c * ROWS : (c + 1) * ROWS], np.float64)
        bw_u16 = bw.view(np.uint16)
        for li, b in enumerate((b1, b2)):
            for m in range(2):
                c = (2 * li + m) * 2
                bw_u16[:, c : c + 2] = (
                    b[m * 128 : (m + 1) * 128].astype(f32).view(np.uint16).reshape(P, 2)
                )
        bw_u16[:, 8:10] = 0
        bw_u16[:, 10:16] = 0
        bw[:, X1_OFF : X1_OFF + 128] = x1s[:, 0:128].T.astype(f16)
        bw[:, X1_OFF + 128 : X1_OFF + 256] = x1s[:, 128:256].T.astype(f16)
        bw[:, W1_OFF:W2_OFF] = w1s
        bw[:, W2_OFF:W3_OFF] = w2s[:, 0:256]
        w3k0_sw = np.concatenate(
            [w3s[:, 128:256], w3s[:, 0:128]], axis=1)      # [lv|mu] k0
        bw[:, W3_OFF:BW_W] = w3k0_sw
        w2k1a = np.ascontiguousarray(w2s[:, 256:512])
        w3k1a = np.concatenate(
            [w3s[:, 384:512], w3s[:, 256:384]], axis=1)    # [lv|mu] k1
        abb = np.empty((P, 256), f32)
        abb[:, 0:ROWS] = (x2s - m1).T.astype(f32)
        abb[:, ROWS : 2 * ROWS] = (0.5 * (x2s * x2s - m2)).T.astype(f32)
        in_maps.append(
            {"blobW": bw, "blobW2K1": w2k1a, "blobW3K1": w3k1a,
             "brow": brow, "blobAB": abb}
        )
    return in_maps


def _run(in_maps, **kwargs):
    global _module_cache
    if _module_cache is None:
        _module_cache = _build_module()
    return run_bass_kernel_spmd(
        _module_cache, in_maps, core_ids=list(range(NCORES)), **kwargs
    )


def _combine(results):
    tot = 0.0
    for r in results:
        o = np.asarray(r["out"], np.float64)
        tot += float(np.sum(o[:, 0] - o[:, 1]))
    return np.float32(tot / N)


def kernel(x1, x2, W1, b1, W2, b2, W3, b3):
    in_maps = _pack_inputs(x1, x2, W1, b1, W2, b2, W3, b3)
    res = _run(in_maps)
    return _combine(res.results)
